# revision 1
# baseline (speedup 1.0000x reference)
"""Trainium2 Bass kernel for nn_AttentionLayer_86629490360750.

reference:
    scores = einsum('bqd,bkd->bqk', query, value)   # no 1/sqrt(d) scaling
    dist   = softmax(scores, axis=-1)
    out    = einsum('bqk,bkd->bqd', dist, value)

Shapes: query/value [4, 4096, 64] fp32.

Sharding: 8 cores; core c handles batch b = c//2, query rows
[h*2048, (h+1)*2048) with h = c%2.  Each core sees its full value[b],
so there are no collectives.  Per-core inputs are laid out on the host
as part of sharding:
  - qt2 [128, 2048]: Q^T duplicated on both partition halves (the PE
    row-group packing below needs lhsT/rhs on the same half),
  - vt2 [128, 2048]: V^T with even kv tiles on partitions 0-63 and odd
    tiles on 64-127 (pair p's columns hold tiles 2p / 2p+1),
  - vs [128, 32, 65]: natural V tiles with a ones column appended (the
    ones column turns the PV matmul into a fused context+denominator
    accumulation).

Per-core algorithm (flash-style, no max subtraction -- scores are
N(0, 64) so |s| < ~55 and exp() stays in fp32 range):
  - for each kv tile pair: S^T tiles = V^T.T @ Q^T as two concurrent
    row-group matmuls (float32r, tile_position=(64,0) for the odd tile),
    exp on ScalarE (PSUM -> SBUF), then accumulate
    ctx^T[65, q] += [V | 1].T @ expS^T (PE, PSUM accumulate).  Row 64 of
    the accumulator is the softmax denominator.
  - tail: transpose ctx^T back to [q, 65] (PE), reciprocal of the
    denominator column + scale (DVE), DMA out.

ScalarE is the bottleneck by hardware necessity: softmax needs
B*SQ*SKV/8 = 8.4M exps per core and exp exists only on ScalarE at
1 elem/cycle/lane (128 x 1.2 GHz); the schedule keeps it ~busy
end-to-end and hides all other engines underneath.
"""

import os
import sys

import numpy as np

for _TRN_REPO in ("/opt/trn_rl_repo", "/root/.axon_site/_ro/trn_rl_repo"):
    if os.path.isdir(_TRN_REPO):
        if _TRN_REPO not in sys.path:
            sys.path.insert(0, _TRN_REPO)
        break

B, SQ, SKV, D = 4, 4096, 4096, 64
NCORES = 8
CORES_PER_B = NCORES // B          # 2
RQ = SQ // CORES_PER_B             # 2048 query rows per core
P = 128
NKT = SKV // P                     # 32 kv tiles
NPAIR = NKT // 2                   # 16 kv tile pairs
QCH = 1024                         # outer q chunk (psum budget)
NOC = RQ // QCH                    # 2
M2 = D + 1                         # 65: V plus a ones column (denominator)
ES_BUFS = 6                        # es pool buffers (sweepable)
NCH_IN = 4                         # input DMA chunks (sweepable)

_CACHE = {}


def _build():
    if "nc" in _CACHE:
        return _CACHE["nc"]

    import concourse.bass as bass  # noqa: F401
    import concourse.mybir as mybir
    import concourse.tile as tile
    from concourse import bacc
    from concourse.masks import make_identity

    f32 = mybir.dt.float32
    f32r = mybir.dt.float32r
    EXP = mybir.ActivationFunctionType.Exp

    nc = bacc.Bacc(
        trn_type="TRN2",
        target_bir_lowering=False,
        debug=False,
        enable_asserts=False,
    )
    qt_d = nc.dram_tensor("qt2", [P, RQ], f32, kind="ExternalInput").ap()
    vt_d = nc.dram_tensor("vt2", [P, NPAIR * P], f32, kind="ExternalInput").ap()
    vs_d = nc.dram_tensor("vs", [P, NKT, M2], f32, kind="ExternalInput").ap()
    o_d = nc.dram_tensor("o", [RQ, D], f32, kind="ExternalOutput").ap()

    with tile.TileContext(nc) as tc:
        with (
            tc.tile_pool(name="const", bufs=1) as const,
            tc.tile_pool(name="sb", bufs=1) as sb,
            tc.tile_pool(name="es", bufs=ES_BUFS) as es_pool,
            tc.tile_pool(name="outp", bufs=4) as out_pool,
            tc.tile_pool(name="acc", bufs=1, space="PSUM") as acc_pool,
            tc.tile_pool(name="st", bufs=3, space="PSUM") as st_pool,
        ):
            ident = const.tile([M2, M2], f32)
            make_identity(nc, ident[:])
            # early PE op: starts the cost-model p-state ramp (and HW
            # pipelining) before the input DMAs land; uses the acc slot,
            # which is idle until the first phase-2 matmul
            warm = acc_pool.tile([M2, M2], f32, tag="acc")
            nc.tensor.transpose(warm[:], ident[:], ident[:])

            qt2 = sb.tile([P, RQ], f32r)
            vt2 = sb.tile([P, NPAIR * P], f32r)
            v_sb = sb.tile([P, NKT, M2], f32r)

            # Contiguous per-partition DMAs, chunked so the first pairs
            # unblock early.
            NCH = NCH_IN
            for h in range(NCH):
                qs = slice(h * (RQ // NCH), (h + 1) * (RQ // NCH))
                vs_ = slice(h * (NPAIR * P // NCH), (h + 1) * (NPAIR * P // NCH))
                nc.sync.dma_start(qt2[:, qs], qt_d[:, qs].bitcast(f32r))
                nc.sync.dma_start(vt2[:, vs_], vt_d[:, vs_].bitcast(f32r))
            for h in range(NCH):
                ks = slice(h * (NKT // NCH), (h + 1) * (NKT // NCH))
                nc.sync.dma_start(
                    v_sb[:, ks, :], vs_d[:, ks, :].bitcast(f32r)
                )

            def make_tail(oc, acc):
                """Emission closures for the oc tail: acc copies, then a
                PE-transpose -> DVE reciprocal+scale pipeline per q tile,
                with the output DMA split in quarters so it starts early."""
                acc_sb = sb.tile([M2, QCH], f32, tag=f"accsb{oc}")
                ot = out_pool.tile([P, QCH // P, D], f32, tag=f"ot{oc}")
                tps = {}
                pieces = []
                NJT = QCH // P

                def cp(quarter):
                    def go():
                        cs = slice(
                            quarter * (QCH // 4), (quarter + 1) * (QCH // 4)
                        )
                        nc.any.tensor_copy(acc_sb[:, cs], acc[:, cs])

                    return go

                def tr_piece(jt):
                    def go():
                        tp = st_pool.tile([P, P], f32, tag="st")
                        nc.tensor.transpose(
                            tp[:, 0:M2],
                            acc_sb[:, jt * P : (jt + 1) * P],
                            ident[:],
                        )
                        tps[jt] = tp

                    return go

                def nm_piece(jt):
                    def go():
                        tp = tps.pop(jt)
                        r = out_pool.tile([P, 1], f32)
                        nc.vector.reciprocal(r[:], tp[:, D : D + 1])
                        nc.vector.tensor_scalar_mul(
                            ot[:, jt, :], tp[:, 0:D], r[:]
                        )

                    return go

                def dma_piece(half):
                    def go():
                        t0 = half * (NJT // 2)
                        t1 = (half + 1) * (NJT // 2)
                        row0 = oc * QCH + t0 * P
                        row1 = oc * QCH + t1 * P
                        nc.sync.dma_start(
                            o_d[row0:row1, :].rearrange(
                                "(t p) d -> p t d", p=P
                            ),
                            ot[:, t0:t1, :],
                        )

                    return go

                for quarter in range(4):
                    pieces.append(cp(quarter))
                    for jt in range(
                        quarter * NJT // 4, (quarter + 1) * NJT // 4
                    ):
                        pieces.append(tr_piece(jt))
                        pieces.append(nm_piece(jt))
                    if quarter % 2 == 1:
                        pieces.append(dma_piece(quarter // 2))
                return pieces

            pending_tail = []
            for oc in range(NOC):
                acc = acc_pool.tile([M2, QCH], f32)

                def phase2(p, es_a, es_b, oc=oc, acc=acc):
                    # weight-major (consecutive matmuls share the stationary
                    # operand) except the last pair, which goes chunk-major
                    # so the tail's acc quarters finalize progressively
                    last = p == NPAIR - 1
                    if last:
                        order = [(0, 0), (1, 0), (0, 1), (1, 1)]
                    else:
                        order = [(0, 0), (0, 1), (1, 0), (1, 1)]
                    for g, j in order:
                        js = slice(j * 512, (j + 1) * 512)
                        nc.tensor.matmul(
                            acc[:, js],
                            v_sb[:, 2 * p + g, :],
                            (es_a if g == 0 else es_b)[:, js],
                            start=(p == 0 and g == 0),
                            stop=(last and g == 1),
                        )

                prev = None
                for p in range(NPAIR):
                    if pending_tail:
                        pending_tail.pop(0)()
                    st_a = st_pool.tile([P, QCH], f32, tag="st")
                    st_b = st_pool.tile([P, QCH], f32, tag="st")
                    # weight-major: both chunks of a row group back-to-back
                    # (stationary operand reuse; groups still overlap)
                    for g, j in [(0, 0), (0, 1), (1, 0), (1, 1)]:
                        qs = slice(
                            oc * QCH + j * 512, oc * QCH + (j + 1) * 512
                        )
                        js = slice(j * 512, (j + 1) * 512)
                        if g == 0:
                            nc.tensor.matmul(
                                st_a[:, js],
                                vt2[0:D, p * P : (p + 1) * P],
                                qt2[0:D, qs],
                                start=True,
                                stop=True,
                            )
                        else:
                            nc.tensor.matmul(
                                st_b[:, js],
                                vt2[D:P, p * P : (p + 1) * P],
                                qt2[D:P, qs],
                                start=True,
                                stop=True,
                                tile_position=(64, 0),
                            )
                    es_a = es_pool.tile([P, QCH], f32r)
                    es_b = es_pool.tile([P, QCH], f32r)
                    nc.scalar.activation(es_a[:], st_a[:], EXP)
                    if oc == NOC - 1 and p == NPAIR - 1:
                        # split the very last exp so the tail's first
                        # phase-2 chunk unblocks half an exp earlier
                        nc.scalar.activation(es_b[:, 0:512], st_b[:, 0:512], EXP)
                        nc.scalar.activation(es_b[:, 512:QCH], st_b[:, 512:QCH], EXP)
                    else:
                        nc.scalar.activation(es_b[:], st_b[:], EXP)
                    if prev is not None:
                        phase2(*prev)
                    prev = (p, es_a, es_b)
                phase2(*prev)
                pending_tail.extend(make_tail(oc, acc))
            for piece in pending_tail:
                piece()

    nc.compile()
    _CACHE["nc"] = nc
    return nc


def _in_maps(query, value):
    """Host-side sharding: slice per core and lay out the transposed /
    duplicated views the kernel streams directly."""
    query = np.asarray(query, dtype=np.float32)
    value = np.asarray(value, dtype=np.float32)
    maps = []
    ones = np.ones((NKT, P, 1), np.float32)
    for c in range(NCORES):
        b, h = c // CORES_PER_B, c % CORES_PER_B
        qt = query[b, h * RQ : (h + 1) * RQ, :].T          # [64, 2048]
        qt2 = np.ascontiguousarray(np.concatenate([qt, qt], axis=0))
        vt = value[b].T                                     # [64, 4096]
        vt2 = np.ascontiguousarray(
            vt.reshape(D, NPAIR, 2, P).transpose(2, 0, 1, 3).reshape(P, -1)
        )
        v3 = value[b].reshape(NKT, P, D)
        vs = np.ascontiguousarray(
            np.concatenate([v3, ones], axis=2).transpose(1, 0, 2)
        )                                                   # [128, 32, 65]
        maps.append({"qt2": qt2, "vt2": vt2, "vs": vs})
    return maps


def run(query, value, trace=False):
    """Returns (output [4, 4096, 64] fp32, BassKernelResults)."""
    nc = _build()
    from concourse.bass_utils import run_bass_kernel_spmd

    res = run_bass_kernel_spmd(
        nc, _in_maps(query, value), core_ids=list(range(NCORES)), trace=trace
    )
    out = np.empty((B, SQ, D), np.float32)
    for c in range(NCORES):
        b, h = c // CORES_PER_B, c % CORES_PER_B
        out[b, h * RQ : (h + 1) * RQ, :] = res.results[c]["o"]
    return out, res


def kernel(query, value):
    out, _ = run(query, value)
    return out



# revision 4
# speedup vs baseline: 1.2798x; 1.2798x over previous
"""Trainium2 Bass kernel for nn_AttentionLayer_86629490360750.

reference:
    scores = einsum('bqd,bkd->bqk', query, value)   # no 1/sqrt(d) scaling
    dist   = softmax(scores, axis=-1)
    out    = einsum('bqk,bkd->bqd', dist, value)

Shapes: query/value [4, 4096, 64] fp32.

Sharding: 8 cores; core c handles batch b = c//2, query rows
[h*2048, (h+1)*2048) with h = c%2.  Each core sees all of value[b], so
there are no collectives.  Host-side layout per core:
  - qt [64, 2048]: Q^T slice (contraction dim on partitions),
  - vt [64, 4096]: V^T (phase-1 stationary tiles),
  - vs [128, 32, 65]: natural V tiles + ones column, in bf16 (the ones
    column makes the PV matmul accumulate the softmax denominator).

Per-core algorithm (no max subtraction: scores are N(0, 64), so exp
stays in fp32/bf16 range):
  phase 1  S^T tile [128 kv, 1024 q] = V^T.T @ Q^T   (PE, f32r)
  exp      es = exp(S^T) in bf16, split across two engines:
             - ScalarE: exact exp activation (bf16 out)
             - DVE: Schraudolph fast-exp -- one tensor_scalar
               int16(s*128/ln2 + b) whose bits read as bf16 give
               exp(s) to ~1.5% elementwise; softmax renormalization
               cancels most of it (measured ~5e-3 output rel err)
  phase 2  ctx[q 128, 65] += es^T(kv,q).T @ [V|1]    (PE, bf16)
             8 accumulators per 1024-q chunk, 65-wide outputs: in the
             cost model a bf16 matmul charges out-free-size cycles
             regardless of contraction depth, so this orientation is
             ~2x cheaper than the ctx^T one and needs no transposes.
  tail     reciprocal of col 64 (DVE) * cols 0..63 (DVE/ScalarE), DMA.

PE is the bottleneck: per core phase 1 streams 65536 columns and
phase 2 33280, ~41us at 2.4GHz; the exp work (64 tiles) is split
~18/14 per chunk between ScalarE (~1.04us/tile) and DVE (~1.19us/tile)
so both stay under PE, and the schedule hides DMA + tail underneath.
"""

import math
import os
import sys

import numpy as np

for _TRN_REPO in ("/opt/trn_rl_repo", "/root/.axon_site/_ro/trn_rl_repo"):
    if os.path.isdir(_TRN_REPO):
        if _TRN_REPO not in sys.path:
            sys.path.insert(0, _TRN_REPO)
        break

B, SQ, SKV, D = 4, 4096, 4096, 64
NCORES = 8
CORES_PER_B = NCORES // B          # 2
RQ = SQ // CORES_PER_B             # 2048 query rows per core
P = 128
NKT = SKV // P                     # 32 kv tiles
QCH = 1024                         # q chunk (psum/exp granularity)
NOC = RQ // QCH                    # 2
M2 = D + 1                         # 65: V plus ones column
NQT = QCH // P                     # 8 q sub-tiles per chunk
ES_BUFS = 6                        # es pool depth (sweepable)
ST_BUFS = 3                        # scores psum depth (sweepable)
NWARM = 10                         # PE ramp warm matmuls (sweepable)

# Schraudolph fast-exp: bits of int16(s*A + B) read as bf16 ~= exp(s).
SCH_A = 128.0 / math.log(2.0)
SCH_B = 127.0 * 128.0 - 3.15

# kv tiles whose exp runs on DVE (fast-exp); rest on ScalarE (exact).
# Odd tiles except the last two, so the end of each chunk leans on the
# exact/faster engine and DVE is free for the tail.
DVE_P = frozenset(p for p in range(1, NKT, 2) if p not in (29, 31))

_CACHE = {}


def _build():
    if "nc" in _CACHE:
        return _CACHE["nc"]

    import concourse.bass as bass  # noqa: F401
    import concourse.mybir as mybir
    import concourse.tile as tile
    from concourse import bacc

    f32 = mybir.dt.float32
    f32r = mybir.dt.float32r
    bf16 = mybir.dt.bfloat16
    i16 = mybir.dt.int16
    EXP = mybir.ActivationFunctionType.Exp
    MULT = mybir.AluOpType.mult
    ADD = mybir.AluOpType.add

    nc = bacc.Bacc(
        trn_type="TRN2",
        target_bir_lowering=False,
        debug=False,
        enable_asserts=False,
    )
    qt_d = nc.dram_tensor("qt", [D, RQ], f32, kind="ExternalInput").ap()
    vt_d = nc.dram_tensor("vt", [D, SKV], f32, kind="ExternalInput").ap()
    vs_d = nc.dram_tensor("vs", [P, NKT, M2], bf16, kind="ExternalInput").ap()
    o_d = nc.dram_tensor("o", [RQ, D], f32, kind="ExternalOutput").ap()

    with tile.TileContext(nc) as tc:
        with (
            tc.tile_pool(name="const", bufs=1) as const,
            tc.tile_pool(name="sb", bufs=1) as sb,
            tc.tile_pool(name="es", bufs=ES_BUFS) as esp,
            tc.tile_pool(name="outp", bufs=2) as outp,
            tc.tile_pool(name="rp", bufs=4) as rp,
            tc.tile_pool(name="st", bufs=ST_BUFS, space="PSUM") as stp,
            tc.tile_pool(name="acc", bufs=2, space="PSUM") as accp,
        ):
            # PE p-state warmup: tiny bf16 matmuls from t~0 keep the PE
            # ramp clock running while the input DMAs land.
            wz = const.tile([P, P], bf16)
            nc.gpsimd.memset(wz[:], 0.0)
            warm = stp.tile([P, QCH], f32, tag="st")
            for w in range(NWARM):
                nc.tensor.matmul(
                    warm[:, (w % 4) * P : (w % 4 + 1) * P],
                    wz[:],
                    wz[:],
                    start=True,
                    stop=True,
                )

            qt = sb.tile([D, RQ], f32r)
            vt = sb.tile([D, SKV], f32r)
            v_sb = sb.tile([P, NKT, M2], bf16)

            # Input DMAs, chunked so the first kv tiles unblock early.
            nc.sync.dma_start(vt[:, 0:512], vt_d[:, 0:512].bitcast(f32r))
            nc.sync.dma_start(qt[:, 0:QCH], qt_d[:, 0:QCH].bitcast(f32r))
            nc.sync.dma_start(v_sb[:, 0:16, :], vs_d[:, 0:16, :])
            nc.sync.dma_start(vt[:, 512:2048], vt_d[:, 512:2048].bitcast(f32r))
            nc.sync.dma_start(v_sb[:, 16:NKT, :], vs_d[:, 16:NKT, :])
            nc.sync.dma_start(vt[:, 2048:SKV], vt_d[:, 2048:SKV].bitcast(f32r))
            nc.sync.dma_start(qt[:, QCH:RQ], qt_d[:, QCH:RQ].bitcast(f32r))

            def make_tail(oc, accs):
                """Per-q-subtile normalize (reciprocal of the denominator
                column, then scale) and the output DMAs, as emission
                closures interleaved into the next chunk's loop."""
                ot = outp.tile([P, NQT, D], f32, tag=f"ot{oc}")
                pieces = []

                def norm_piece(qi):
                    def go():
                        h, sl = qi // 4, (qi % 4) * P
                        acc = accs[h]
                        r = rp.tile([P, 1], f32, tag="r")
                        nc.vector.reciprocal(r[:], acc[:, sl + D : sl + D + 1])
                        if qi % 2 == 0:
                            nc.vector.tensor_scalar_mul(
                                ot[:, qi, :], acc[:, sl : sl + D], r[:]
                            )
                        else:
                            nc.scalar.mul(ot[:, qi, :], acc[:, sl : sl + D], r[:])

                    return go

                def dma_piece(half):
                    def go():
                        t0, t1 = half * (NQT // 2), (half + 1) * (NQT // 2)
                        row0 = oc * QCH + t0 * P
                        row1 = oc * QCH + t1 * P
                        nc.sync.dma_start(
                            o_d[row0:row1, :].rearrange("(t p) d -> p t d", p=P),
                            ot[:, t0:t1, :],
                        )

                    return go

                for qi in range(NQT):
                    pieces.append(norm_piece(qi))
                    if qi % (NQT // 2) == NQT // 2 - 1:
                        pieces.append(dma_piece(qi // (NQT // 2)))
                return pieces

            pending_tail = []
            for oc in range(NOC):
                accs = [
                    accp.tile([P, 4 * P], f32, tag="acc", name=f"acc{oc}_{h}")
                    for h in range(2)
                ]

                def phase2(p, es, accs=accs, oc=oc):
                    # 4 accumulators share each psum bank ("zero region"):
                    # only the bank's first matmul starts the group (marking
                    # the whole region pending-zero; siblings fresh-write),
                    # and only its last one stops it.
                    for qi in range(NQT):
                        h, sl = qi // 4, (qi % 4) * P
                        nc.tensor.matmul(
                            accs[h][:, sl : sl + M2],
                            es[:, qi * P : (qi + 1) * P].bitcast(bf16),
                            v_sb[:, p, :],
                            start=(p == 0 and qi % 4 == 0),
                            stop=(p == NKT - 1 and qi % 4 == 3),
                        )

                prev = None
                for p in range(NKT):
                    if pending_tail:
                        pending_tail.pop(0)()
                    st = stp.tile([P, QCH], f32, tag="st")
                    for j in range(2):
                        nc.tensor.matmul(
                            st[:, j * 512 : (j + 1) * 512],
                            vt[:, p * P : (p + 1) * P],
                            qt[:, oc * QCH + j * 512 : oc * QCH + (j + 1) * 512],
                            start=True,
                            stop=True,
                        )
                    es = esp.tile([P, QCH], i16, tag="es")
                    if p in DVE_P:
                        nc.vector.tensor_scalar(
                            es[:], st[:], SCH_A, SCH_B, MULT, ADD
                        )
                    else:
                        nc.scalar.activation(es[:].bitcast(bf16), st[:], EXP)
                    if prev is not None:
                        phase2(*prev)
                    prev = (p, es)
                phase2(*prev)
                pending_tail.extend(make_tail(oc, accs))
            for piece in pending_tail:
                piece()

    nc.compile()
    _CACHE["nc"] = nc
    return nc


def _in_maps(query, value):
    """Host-side sharding: slice per core into the layouts the kernel
    streams directly (transposes + bf16 V tiles with ones column)."""
    import ml_dtypes

    query = np.asarray(query, dtype=np.float32)
    value = np.asarray(value, dtype=np.float32)
    maps = []
    ones = np.ones((NKT, P, 1), np.float32)
    for c in range(NCORES):
        b, h = c // CORES_PER_B, c % CORES_PER_B
        qt = np.ascontiguousarray(query[b, h * RQ : (h + 1) * RQ, :].T)
        vt = np.ascontiguousarray(value[b].T)
        v3 = value[b].reshape(NKT, P, D)
        vs = np.ascontiguousarray(
            np.concatenate([v3, ones], axis=2)
            .transpose(1, 0, 2)
            .astype(ml_dtypes.bfloat16)
        )
        maps.append({"qt": qt, "vt": vt, "vs": vs})
    return maps


def run(query, value, trace=False):
    """Returns (output [4, 4096, 64] fp32, BassKernelResults)."""
    nc = _build()
    from concourse.bass_utils import run_bass_kernel_spmd

    res = run_bass_kernel_spmd(
        nc, _in_maps(query, value), core_ids=list(range(NCORES)), trace=trace
    )
    out = np.empty((B, SQ, D), np.float32)
    for c in range(NCORES):
        b, h = c // CORES_PER_B, c % CORES_PER_B
        out[b, h * RQ : (h + 1) * RQ, :] = res.results[c]["o"]
    return out, res


def kernel(query, value):
    out, _ = run(query, value)
    return out
